# revision 15
# baseline (speedup 1.0000x reference)
"""Trainium2 Bass kernel for nn_BertPooler (binarized BertPooler head).

Math (see reference):
    x   = hidden_states[:, 0, :]                      # [B, H] first token
    xq  = sign(x) * max(alpha, 1e-5)
    wq  = sign(W) * mean(|W|)
    y   = tanh(xq @ wq.T + b)                         # [B, 1, H]

Sharding (8 cores):
  - Output features o are sharded 128 per core. Core c computes
    y[:, 0, 128c:128c+128].
  - Each core receives the FULL weight matrix (rolled so that its own
    128 output rows come first) because mean(|W|) is a global reduction;
    the 4 MB W load is the per-core memory roofline.
  - hidden_states is sliced to the first token on the host (pure data
    movement); the 128 MB bulk tensor is never touched by the device.

Per-core device program:
  - Small inputs (x^T, bias, alpha) DMA on the scalar HWDGE ring so they
    are not queued behind the 4 MB W load on the sync ring.
  - W arrives in 5 chunks (0.5/1/1/1/0.5 MB); DVE abs-reduces each as it
    lands (-> mean|W|). Small first chunk = the matmul shard (early
    sign/transpose); small last chunk shortens the tail reduce.
  - ACT sign of shard + x^T (bf16), 8 PE transposes, 1 big PSUM->SBUF
    copy, 8 accumulating PE matmuls: S[o,b] = sum_h sg(W)[o,h] sg(x)[b,h].
  - Partition-broadcast of (sum|W|, clamped alpha) via a ones-matmul,
    ordered after the main matmuls so it doesn't stall them.
  - One ACT instruction: y = tanh(S * (alpha*mean|W|) + b_shard).
All arithmetic of the reference runs on device; the host only
slices/permutes inputs and reassembles the output.
"""

import os
import sys

import numpy as np

sys.path.insert(0, "/opt/trn_rl_repo")

import concourse.bass as bass  # noqa: E402
import concourse.mybir as mybir  # noqa: E402
from concourse import bacc  # noqa: E402
from concourse.bass_utils import run_bass_kernel_spmd  # noqa: E402
from concourse.masks import make_identity  # noqa: E402
from concourse.tile import TileContext  # noqa: E402
from concourse.tile_rust import add_dep_helper  # noqa: E402


def _ensure_axon_ntff_hook():
    """Register the axon NTFF profiling hook if the image's antenv lacks
    the antenv.axon_hooks registration channel. Without this, running
    with BASS_TRACE=1 raises ModuleNotFoundError in bass_utils; with it,
    tracing works (or degrades gracefully if the .so is too old)."""
    try:
        import antenv.axon_hooks  # noqa: F401

        return
    except ImportError:
        pass
    try:
        import types

        import antenv

        mod = types.ModuleType("antenv.axon_hooks")
        mod._hook = None

        def set_axon_ntff_profile_hook(h):
            mod._hook = h

        def get_axon_ntff_profile_hook():
            return mod._hook

        mod.set_axon_ntff_profile_hook = set_axon_ntff_profile_hook
        mod.get_axon_ntff_profile_hook = get_axon_ntff_profile_hook
        sys.modules["antenv.axon_hooks"] = mod
        antenv.axon_hooks = mod

        from trn_agent_boot.trn_boot import _ntff_profile_via_ctypes

        so_path = "/opt/axon/libaxon_pjrt.so"
        if os.path.exists(so_path):
            hook = _ntff_profile_via_ctypes(so_path)
            if hook is not None:
                set_axon_ntff_profile_hook(hook)
    except Exception:
        pass


_ensure_axon_ntff_hook()

B, S, H = 8, 4096, 1024
NCORES = 8
OSH = H // NCORES  # 128 output features per core
EPS = 1e-5

_NC = None
LAST_RESULTS = None


def _raw(inst):
    return getattr(inst, "ins", inst)


def _build():
    # Bacc (not plain Bass): its compile() pass pipeline splits multi-sem
    # waits into event semaphores — TRN2 allows only 1 wait per instruction.
    nc = bacc.Bacc(None, enable_partition_id=False)
    f32 = mybir.dt.float32
    bf16 = mybir.dt.bfloat16

    Wp = nc.dram_tensor("Wp", [H, H], f32, kind="ExternalInput")
    xT = nc.dram_tensor("xT", [128, 64], f32, kind="ExternalInput")
    bsh = nc.dram_tensor("bsh", [OSH, 1], f32, kind="ExternalInput")
    alpha = nc.dram_tensor("alpha", [1, 1], f32, kind="ExternalInput")
    yT = nc.dram_tensor("yT", [OSH, B], f32, kind="ExternalOutput")

    with TileContext(nc) as tc:
        with (
            tc.tile_pool(name="w", bufs=5) as wpool,
            tc.tile_pool(name="s", bufs=1) as spool,
            tc.tile_pool(name="ptp", bufs=1, space="PSUM") as ptp,
            tc.tile_pool(name="pacc", bufs=1, space="PSUM") as pacc,
        ):
            # ---- small inputs on the scalar HWDGE ring (independent of
            # the big W transfers on the sync ring) ----
            xt = spool.tile([128, 64], f32)
            nc.scalar.dma_start(out=xt[:], in_=xT[:])
            al = spool.tile([1, 1], f32)
            nc.scalar.dma_start(out=al[:], in_=alpha[:])
            bsb = spool.tile([OSH, 1], f32)
            nc.scalar.dma_start(out=bsb[:], in_=bsh[:])

            # ---- identity for PE transpose, built on gpsimd ----
            idt = spool.tile([128, 128], bf16)
            make_identity(nc, idt[:])

            # ---- W load: 5 chunks (rows 0-127, 128-383, 384-639,
            # 640-895, 896-1023) ----
            wsh = wpool.tile([128, 1024], f32, tag="wsh")
            nc.sync.dma_start(out=wsh[:], in_=Wp[0:128, :])
            wmids = []
            for t in range(3):
                wm = wpool.tile([128, 2, 1024], f32, tag="wmid")
                src = Wp[128 + 256 * t : 128 + 256 * (t + 1), :].rearrange(
                    "(two p) h -> p two h", p=128
                )
                nc.sync.dma_start(out=wm[:], in_=src)
                wmids.append(wm)
            wlast = wpool.tile([128, 1024], f32, tag="wlast")
            nc.sync.dma_start(out=wlast[:], in_=Wp[896:1024, :])

            # ---- sign of x^T and of this core's W shard (bf16) ----
            sx = spool.tile([128, 64], bf16)
            nc.scalar.activation(sx[:], xt[:], mybir.ActivationFunctionType.Sign)
            sw = spool.tile([128, 1024], bf16)
            nc.scalar.activation(sw[:], wsh[:], mybir.ActivationFunctionType.Sign)

            # ---- abs partial sums, one col per chunk ----
            parts = spool.tile([128, 5], f32)
            nc.vector.tensor_reduce(
                out=parts[:, 0:1],
                in_=wsh[:],
                axis=mybir.AxisListType.X,
                op=mybir.AluOpType.add,
                apply_absolute_value=True,
            )
            for t in range(3):
                nc.vector.tensor_reduce(
                    out=parts[:, 1 + t : 2 + t],
                    in_=wmids[t][:],
                    axis=mybir.AxisListType.XY,
                    op=mybir.AluOpType.add,
                    apply_absolute_value=True,
                )
            nc.vector.tensor_reduce(
                out=parts[:, 4:5],
                in_=wlast[:],
                axis=mybir.AxisListType.X,
                op=mybir.AluOpType.add,
                apply_absolute_value=True,
            )

            # ---- alpha: clamp, stage into bcast rhs ----
            alc = spool.tile([1, 1], f32)
            nc.vector.tensor_scalar_max(alc[:], al[:], EPS)
            rhs_bc = spool.tile([128, 2], f32)
            nc.vector.memset(rhs_bc[:, 1:2], 0.0)
            nc.vector.tensor_copy(rhs_bc[0:1, 1:2], alc[:])
            nc.vector.tensor_reduce(
                out=rhs_bc[:, 0:1],
                in_=parts[:],
                axis=mybir.AxisListType.X,
                op=mybir.AluOpType.add,
            )

            # ---- transpose shard blocks: sw [o,h] -> swt chunks [h,o] ----
            tp_all = ptp.tile([128, 8, 128], bf16)  # one PSUM bank
            for hc in range(8):
                nc.tensor.transpose(
                    tp_all[:, hc, :], sw[:, 128 * hc : 128 * (hc + 1)], idt[:]
                )
            swt_all = spool.tile([128, 8, 128], bf16)
            nc.vector.tensor_copy(swt_all[:], tp_all[:])

            # ---- S[o, b] = sum_h sign(W)[o, h] * sign(x)[b, h] ----
            s_ps = pacc.tile([128, B], f32)
            mm_last = None
            for hc in range(8):
                mm_last = nc.tensor.matmul(
                    s_ps[:],
                    swt_all[:, hc, :],
                    sx[:, B * hc : B * (hc + 1)],
                    start=(hc == 0),
                    stop=(hc == 7),
                )

            # ---- broadcast (sum|W|, alpha_c) to all partitions ----
            ones = spool.tile([128, 128], f32)
            nc.vector.memset(ones[:], 1.0)
            bc_ps = pacc.tile([128, 2], f32)
            bc_mm = nc.tensor.matmul(bc_ps[:], ones[:], rhs_bc[:], start=True, stop=True)
            # The bcast matmul is only ready after the full |W| reduction;
            # keep it behind the early-ready main matmuls in PE order.
            add_dep_helper(
                _raw(bc_mm), _raw(mm_last), sync=False, reason="bc after mms"
            )

            # scale = alpha_c * sum|W| / (H*H)
            scale = spool.tile([128, 1], f32)
            nc.vector.tensor_scalar(
                out=scale[:],
                in0=bc_ps[:, 0:1],
                scalar1=bc_ps[:, 1:2],
                scalar2=1.0 / (H * H),
                op0=mybir.AluOpType.mult,
                op1=mybir.AluOpType.mult,
            )

            # ---- y^T = tanh(S * scale + b), one ACT instruction;
            # output DMA issued from the same engine (no extra sem hop) ----
            ysb = spool.tile([OSH, B], f32)
            nc.scalar.activation(
                ysb[:],
                s_ps[:],
                mybir.ActivationFunctionType.Tanh,
                bias=bsb[:],
                scale=scale[:],
            )
            nc.scalar.dma_start(out=yT[:], in_=ysb[:])

    nc.compile()
    return nc


def _get_nc():
    global _NC
    if _NC is None:
        _NC = _build()
    return _NC


def kernel(hidden_states, W, b, alpha):
    global LAST_RESULTS
    hidden_states = np.asarray(hidden_states, dtype=np.float32)
    W = np.asarray(W, dtype=np.float32)
    b = np.asarray(b, dtype=np.float32)
    alpha = np.asarray(alpha, dtype=np.float32)

    # Host-side data movement only: slice first token, transpose layout.
    x = np.ascontiguousarray(hidden_states[:, 0, :])  # [B, H]
    # xTl[p, hc*8 + b] = x[b, hc*128 + p]
    xTl = np.ascontiguousarray(
        x.reshape(B, 8, 128).transpose(2, 1, 0).reshape(128, 64)
    )
    alpha2 = alpha.reshape(1, 1)

    in_maps = []
    for c in range(NCORES):
        Wp = np.ascontiguousarray(np.roll(W, -OSH * c, axis=0))
        in_maps.append(
            {
                "Wp": Wp,
                "xT": xTl,
                "bsh": np.ascontiguousarray(b[OSH * c : OSH * (c + 1)]).reshape(
                    OSH, 1
                ),
                "alpha": alpha2,
            }
        )

    nc = _get_nc()
    res = run_bass_kernel_spmd(nc, in_maps, core_ids=list(range(NCORES)))
    LAST_RESULTS = res

    out = np.empty((B, 1, H), dtype=np.float32)
    for c in range(NCORES):
        out[:, 0, OSH * c : OSH * (c + 1)] = res.results[c]["yT"].T
    return out


# revision 21
# speedup vs baseline: 1.0358x; 1.0358x over previous
"""Trainium2 Bass kernel for nn_BertPooler (binarized BertPooler head).

Math (see reference):
    x   = hidden_states[:, 0, :]                      # [B, H] first token
    xq  = sign(x) * max(alpha, 1e-5)
    wq  = sign(W) * mean(|W|)
    y   = tanh(xq @ wq.T + b)                         # [B, 1, H]

Sharding (8 cores):
  - Output features o are sharded 128 per core. Core c computes
    y[:, 0, 128c:128c+128].
  - Each core receives the FULL weight matrix (rolled so that its own
    128 output rows come first) because mean(|W|) is a global reduction;
    the 4 MB W load is the per-core memory roofline.
  - hidden_states is sliced to the first token on the host (pure data
    movement); the 128 MB bulk tensor is never touched by the device.

Per-core device program:
  - Small inputs (x^T, bias, alpha) DMA on the scalar HWDGE ring so they
    are not queued behind the 4 MB W load on the sync ring.
  - W arrives in 5 chunks (0.5/1/1/1/0.5 MB); DVE abs-reduces each as it
    lands (-> mean|W|). Small first chunk = the matmul shard (early
    sign/transpose); small last chunk shortens the tail reduce.
  - ACT sign of shard + x^T (bf16), 8 PE transposes, 1 big PSUM->SBUF
    copy, 8 accumulating PE matmuls: S[o,b] = sum_h sg(W)[o,h] sg(x)[b,h].
  - Partition-broadcast of (sum|W|, clamped alpha) via a ones-matmul,
    ordered after the main matmuls so it doesn't stall them.
  - One ACT instruction: y = tanh(S * (alpha*mean|W|) + b_shard).
All arithmetic of the reference runs on device; the host only
slices/permutes inputs and reassembles the output.
"""

import os
import sys

import numpy as np

sys.path.insert(0, "/opt/trn_rl_repo")

import concourse.bass as bass  # noqa: E402
import concourse.mybir as mybir  # noqa: E402
from concourse import bacc  # noqa: E402
from concourse.bass_utils import run_bass_kernel_spmd  # noqa: E402
from concourse.tile import TileContext  # noqa: E402
from concourse.tile_rust import add_dep_helper  # noqa: E402


def _ensure_axon_ntff_hook():
    """Register the axon NTFF profiling hook if the image's antenv lacks
    the antenv.axon_hooks registration channel. Without this, running
    with BASS_TRACE=1 raises ModuleNotFoundError in bass_utils; with it,
    tracing works (or degrades gracefully if the .so is too old)."""
    try:
        import antenv.axon_hooks  # noqa: F401

        return
    except ImportError:
        pass
    try:
        import types

        import antenv

        mod = types.ModuleType("antenv.axon_hooks")
        mod._hook = None

        def set_axon_ntff_profile_hook(h):
            mod._hook = h

        def get_axon_ntff_profile_hook():
            return mod._hook

        mod.set_axon_ntff_profile_hook = set_axon_ntff_profile_hook
        mod.get_axon_ntff_profile_hook = get_axon_ntff_profile_hook
        sys.modules["antenv.axon_hooks"] = mod
        antenv.axon_hooks = mod

        from trn_agent_boot.trn_boot import _ntff_profile_via_ctypes

        so_path = "/opt/axon/libaxon_pjrt.so"
        if os.path.exists(so_path):
            hook = _ntff_profile_via_ctypes(so_path)
            if hook is not None:
                set_axon_ntff_profile_hook(hook)
    except Exception:
        pass


_ensure_axon_ntff_hook()

B, S, H = 8, 4096, 1024
NCORES = 8
OSH = H // NCORES  # 128 output features per core
EPS = 1e-5

_NC = None
LAST_RESULTS = None


def _raw(inst):
    return getattr(inst, "ins", inst)


def _build():
    # Bacc (not plain Bass): its compile() pass pipeline splits multi-sem
    # waits into event semaphores — TRN2 allows only 1 wait per instruction.
    nc = bacc.Bacc(None, enable_partition_id=False)
    f32 = mybir.dt.float32
    bf16 = mybir.dt.bfloat16

    Wp = nc.dram_tensor("Wp", [H, H], f32, kind="ExternalInput")
    # Packed small inputs: cols 0:64 = x^T layout, col 64 = bias shard,
    # col 65 = alpha (host-replicated per partition). One clean DMA with
    # 264 contiguous bytes per partition instead of three fine-grained
    # transfers that steal SDMA cycles from the W stream.
    sm = nc.dram_tensor("sm", [128, 66], f32, kind="ExternalInput")
    yT = nc.dram_tensor("yT", [OSH, B], f32, kind="ExternalOutput")
    import ml_dtypes

    id_dram = nc.inline_tensor(np.eye(128, dtype=ml_dtypes.bfloat16), name="id128")

    with TileContext(nc) as tc:
        with (
            tc.tile_pool(name="w", bufs=6) as wpool,
            tc.tile_pool(name="s", bufs=1) as spool,
            tc.tile_pool(name="ptp", bufs=1, space="PSUM") as ptp,
            tc.tile_pool(name="pacc", bufs=1, space="PSUM") as pacc,
        ):
            # ---- small inputs on the scalar HWDGE ring (independent of
            # the big W transfers on the sync ring) ----
            smt = spool.tile([128, 66], f32)
            nc.scalar.dma_start(out=smt[:], in_=sm[:])
            # identity for PE transpose (inline const: loads during the
            # fixed NEFF startup window via the static weight path)
            idt = spool.tile([128, 128], bf16)
            nc.scalar.dma_start(out=idt[:], in_=id_dram[:])

            # ---- W load: 6 chunks (rows 0-127, 128-383, 384-639,
            # 640-895, then rows 896-1023 split into two column halves
            # so the tail abs-reduce is short) ----
            wsh = wpool.tile([128, 1024], f32, tag="wsh")
            nc.sync.dma_start(out=wsh[:], in_=Wp[0:128, :])
            wmids = []
            for t in range(3):
                wm = wpool.tile([128, 2, 1024], f32, tag="wmid")
                src = Wp[128 + 256 * t : 128 + 256 * (t + 1), :].rearrange(
                    "(two p) h -> p two h", p=128
                )
                nc.sync.dma_start(out=wm[:], in_=src)
                wmids.append(wm)
            wl0 = wpool.tile([128, 512], f32, tag="wl0")
            nc.sync.dma_start(out=wl0[:], in_=Wp[896:1024, 0:512])
            wl1 = wpool.tile([128, 512], f32, tag="wl1")
            nc.sync.dma_start(out=wl1[:], in_=Wp[896:1024, 512:1024])

            # ---- sign of x^T and of this core's W shard (bf16) ----
            sx = spool.tile([128, 64], bf16)
            nc.scalar.activation(sx[:], smt[:, 0:64], mybir.ActivationFunctionType.Sign)
            sw = spool.tile([128, 1024], bf16)
            nc.scalar.activation(sw[:], wsh[:], mybir.ActivationFunctionType.Sign)

            # ---- abs partial sums, one col per chunk ----
            parts = spool.tile([128, 6], f32)
            nc.vector.tensor_reduce(
                out=parts[:, 0:1],
                in_=wsh[:],
                axis=mybir.AxisListType.X,
                op=mybir.AluOpType.add,
                apply_absolute_value=True,
            )
            for t in range(3):
                nc.vector.tensor_reduce(
                    out=parts[:, 1 + t : 2 + t],
                    in_=wmids[t][:],
                    axis=mybir.AxisListType.XY,
                    op=mybir.AluOpType.add,
                    apply_absolute_value=True,
                )
            for t, wl in enumerate((wl0, wl1)):
                nc.vector.tensor_reduce(
                    out=parts[:, 4 + t : 5 + t],
                    in_=wl[:],
                    axis=mybir.AxisListType.X,
                    op=mybir.AluOpType.add,
                    apply_absolute_value=True,
                )

            # ---- alpha clamp (already per-partition) + total abs sum ----
            alc = spool.tile([128, 1], f32)
            nc.vector.tensor_scalar_max(alc[:], smt[:, 65:66], EPS)
            rhs_bc = spool.tile([128, 1], f32)
            nc.vector.tensor_reduce(
                out=rhs_bc[:, 0:1],
                in_=parts[:],
                axis=mybir.AxisListType.X,
                op=mybir.AluOpType.add,
            )

            # ---- transpose shard blocks: sw [o,h] -> swt chunks [h,o] ----
            tp_all = ptp.tile([128, 8, 128], bf16)  # one PSUM bank
            for hc in range(8):
                nc.tensor.transpose(
                    tp_all[:, hc, :], sw[:, 128 * hc : 128 * (hc + 1)], idt[:]
                )
            swt_all = spool.tile([128, 8, 128], bf16)
            nc.vector.tensor_copy(swt_all[:], tp_all[:])

            # ---- S[o, b] = sum_h sign(W)[o, h] * sign(x)[b, h] ----
            s_ps = pacc.tile([128, B], f32)
            mm_last = None
            for hc in range(8):
                mm_last = nc.tensor.matmul(
                    s_ps[:],
                    swt_all[:, hc, :],
                    sx[:, B * hc : B * (hc + 1)],
                    start=(hc == 0),
                    stop=(hc == 7),
                )

            # ---- broadcast sum|W| to all partitions via ones-matmul ----
            ones = spool.tile([128, 128], f32)
            nc.vector.memset(ones[:], 1.0)
            bc_ps = pacc.tile([128, 1], f32)
            bc_mm = nc.tensor.matmul(bc_ps[:], ones[:], rhs_bc[:], start=True, stop=True)
            # The bcast matmul is only ready after the full |W| reduction;
            # keep it behind the early-ready main matmuls in PE order.
            add_dep_helper(
                _raw(bc_mm), _raw(mm_last), sync=False, reason="bc after mms"
            )

            # scale = alpha_c * sum|W| / (H*H)
            scale = spool.tile([128, 1], f32)
            nc.vector.tensor_scalar(
                out=scale[:],
                in0=bc_ps[:, 0:1],
                scalar1=alc[:],
                scalar2=1.0 / (H * H),
                op0=mybir.AluOpType.mult,
                op1=mybir.AluOpType.mult,
            )

            # ---- y^T = tanh(S * scale + b), one ACT instruction;
            # output DMA issued from the same engine (no extra sem hop) ----
            ysb = spool.tile([OSH, B], f32)
            nc.scalar.activation(
                ysb[:],
                s_ps[:],
                mybir.ActivationFunctionType.Tanh,
                bias=smt[:, 64:65],
                scale=scale[:],
            )
            nc.scalar.dma_start(out=yT[:], in_=ysb[:])

    nc.compile()
    return nc


def _get_nc():
    global _NC
    if _NC is None:
        _NC = _build()
    return _NC


def kernel(hidden_states, W, b, alpha):
    global LAST_RESULTS
    hidden_states = np.asarray(hidden_states, dtype=np.float32)
    W = np.asarray(W, dtype=np.float32)
    b = np.asarray(b, dtype=np.float32)
    alpha = np.asarray(alpha, dtype=np.float32)

    # Host-side data movement only: slice first token, transpose layout,
    # pack the small operands into one [128, 66] tensor.
    x = np.ascontiguousarray(hidden_states[:, 0, :])  # [B, H]
    # xTl[p, hc*8 + b] = x[b, hc*128 + p]
    xTl = x.reshape(B, 8, 128).transpose(2, 1, 0).reshape(128, 64)

    in_maps = []
    for c in range(NCORES):
        Wp = np.ascontiguousarray(np.roll(W, -OSH * c, axis=0))
        sm = np.empty((128, 66), dtype=np.float32)
        sm[:, 0:64] = xTl
        sm[:, 64] = b[OSH * c : OSH * (c + 1)]
        sm[:, 65] = alpha[0]
        in_maps.append({"Wp": Wp, "sm": sm})

    nc = _get_nc()
    res = run_bass_kernel_spmd(nc, in_maps, core_ids=list(range(NCORES)))
    LAST_RESULTS = res

    out = np.empty((B, 1, H), dtype=np.float32)
    for c in range(NCORES):
        out[:, 0, OSH * c : OSH * (c + 1)] = res.results[c]["yT"].T
    return out
